# revision 13
# baseline (speedup 1.0000x reference)
"""Trainium2 Bass kernel v3 for nn_CINComp.

out[b,o,d] = sum_{i,j} W[o,i*64+j]*feature[b,i,d]*base[b,j,d] + bias[o]

Data-parallel over batch B=1024 across 8 cores (BLOC=128 b/core).

v3 design ("dup-layout"): chunk the ij=4096 contraction into 32 chunks of
128 = (8 i's x 16 j's). Host-side duplicates BOTH factors to the chunk
partition layout p=(a,b): ftd dups each f-row x16 (8 MB/core), gtd dups
each g-row x8 (4 MB/core). The P=f*g product is then ONE unit-stride
bf16 SBUF DVE tensor_mul per (bd-quarter, i-block) at 2x mode -- no PE
broadcast matmuls, no PSUM intermediate, no ScalarE casts (vs v2 which
spent ~100us PE + ~90us ACT on that). PE does only the 256 main
contraction matmuls (~55us); DVE (~70us) is the pacer.

  - bd=4096 per core split into 4 quarters of 1024; PSUM = 8 banks = 4
    quarters x 2 accumulators [128,512], drained (bias add + bf16 cast)
    on ScalarE per quarter.
  - ~15 MB/core HBM traffic streamed in h-major slices so compute
    starts after ~1.3 MB.
  - dummy matmuls on scratch tiles during the DMA ramp keep/get the PE
    HAM clock warm (2.4 GHz) before the first real matmul.
"""

import numpy as np
import ml_dtypes

import concourse.bass as bass
import concourse.mybir as mybir
import concourse.tile as tile
from concourse.bass_utils import run_bass_kernel_spmd

B, HK, H0, D, O = 1024, 64, 64, 32, 128
NCORES = 8
BLOC = B // NCORES          # 128 batches per core
BD = BLOC * D               # 4096 bd points per core
WIDTHS = [512, 1536, 1536, 512]   # bd split; 512-multiples, sum=BD,
NQ = len(WIDTHS)                  # sum(w//512)=8 PSUM banks
IB = 8                      # i-blocks (8 i's each) -> ftd dup x16
JB = 4                      # j-blocks (16 j's each) -> gtd dup x8
NCHUNK = IB * JB            # 32 chunks of 128 = (a,b) partitions
NWARM = 6                   # PE warm-up matmuls during DMA ramp
GP_OFFLOAD = True           # GpSimd computes ib=7's products per quarter
F32 = mybir.dt.float32
BF16 = mybir.dt.bfloat16
BF = ml_dtypes.bfloat16

_CACHE = {}


def _strip_self_waits(nc: bass.Bass) -> None:
    """Transitively-minimal semaphore waits (see v2 docstring)."""
    UPD = ("sem-inc", "sem-add-imm")
    insts = [i for bb in nc.m.functions[0].blocks for i in bb.instructions]

    bad_sems = set()
    for i in insts:
        si = getattr(i, "sync_info", None)
        if si is None:
            continue
        for u in si.on_update:
            if u.sync_type != "semaphore" or u.update_mode not in UPD:
                bad_sems.add(u.id)

    def fifo_of(i):
        si = i.sync_info
        eng = str(getattr(i, "engine", None))
        if type(i).__name__ == "InstDMACopy" and si is not None:
            for u in si.on_update:
                if u.sync_type == "semaphore" and u.update_mode in UPD:
                    return ("q", u.id)
        return ("e", eng)

    cum: dict = {}
    event: dict = {}
    fifo_pred: dict = {}
    last_in_fifo: dict = {}
    metas = []
    for idx, i in enumerate(insts):
        si = getattr(i, "sync_info", None)
        f = fifo_of(i)
        fifo_pred[idx] = last_in_fifo.get(f)
        last_in_fifo[f] = idx
        ups = []
        if si is not None:
            for u in si.on_update:
                if u.sync_type == "semaphore" and u.update_mode in UPD:
                    cum[u.id] = cum.get(u.id, 0) + u.update_value
                    event[(u.id, cum[u.id])] = idx
                    ups.append((u.id, cum[u.id]))
        metas.append((si, ups))

    def resolve(sem, k):
        v = k
        while (sem, v) not in event:
            v += 1
            if v > cum.get(sem, 0):
                return None
        return event[(sem, v)]

    cvc: list = [None] * len(insts)

    def get_cvc(idx):
        if cvc[idx] is not None:
            return cvc[idx]
        stack = [idx]
        while stack:
            j = stack[-1]
            if cvc[j] is not None:
                stack.pop()
                continue
            si, ups = metas[j]
            deps = []
            p = fifo_pred[j]
            if p is not None:
                deps.append(p)
            if si is not None:
                for w in si.on_wait:
                    if (
                        w.sync_type == "semaphore"
                        and w.wait_mode == "sem-ge-imm"
                        and w.id not in bad_sems
                    ):
                        e = resolve(w.id, w.wait_value)
                        if e is not None and e != j:
                            deps.append(e)
            pending = [d for d in deps if cvc[d] is None]
            if pending:
                stack.extend(pending)
                continue
            stack.pop()
            vc: dict = {}
            for d in deps:
                for s, v in cvc[d].items():
                    if vc.get(s, 0) < v:
                        vc[s] = v
            if si is not None:
                for w in si.on_wait:
                    if (
                        w.sync_type == "semaphore"
                        and w.wait_mode == "sem-ge-imm"
                        and w.id not in bad_sems
                    ):
                        if vc.get(w.id, 0) < w.wait_value:
                            vc[w.id] = w.wait_value
            for s, v in ups:
                if vc.get(s, 0) < v:
                    vc[s] = v
            cvc[j] = vc
        return cvc[idx]

    for idx, i in enumerate(insts):
        si, _ups = metas[idx]
        if si is None or not si.on_wait:
            continue
        base: dict = {}
        p = fifo_pred[idx]
        if p is not None:
            base = dict(get_cvc(p))
        sem_waits = [
            w
            for w in si.on_wait
            if w.sync_type == "semaphore"
            and w.wait_mode == "sem-ge-imm"
            and w.id not in bad_sems
        ]
        other = [w for w in si.on_wait if w not in sem_waits]

        def strength(w):
            e = resolve(w.id, w.wait_value)
            return len(get_cvc(e)) if e is not None else 0

        sem_waits.sort(key=strength, reverse=True)

        def wait_cvc(w):
            e = resolve(w.id, w.wait_value)
            vc = dict(get_cvc(e)) if e is not None else {}
            if vc.get(w.id, 0) < w.wait_value:
                vc[w.id] = w.wait_value
            return vc

        kept = sem_waits[:]
        changed = True
        while changed:
            changed = False
            for w in kept:
                cover = dict(base)
                for w2 in kept:
                    if w2 is w:
                        continue
                    for s, v in wait_cvc(w2).items():
                        if cover.get(s, 0) < v:
                            cover[s] = v
                if cover.get(w.id, 0) >= w.wait_value:
                    kept.remove(w)
                    changed = True
                    break
        if len(kept) + len(other) != len(si.on_wait):
            si.on_wait = other + kept


def _build_nc(strip: bool = True) -> bass.Bass:
    nc = bass.Bass()
    ftd = nc.dram_tensor("ftd", [128, IB, BD], BF16, kind="ExternalInput")
    gtd = nc.dram_tensor("gtd", [128, JB, BD], BF16, kind="ExternalInput")
    wt = nc.dram_tensor("wt", [128, NCHUNK, 128], BF16, kind="ExternalInput")
    bias = nc.dram_tensor("bias", [128, 1], F32, kind="ExternalInput")
    out = nc.dram_tensor("out", [128, BD], BF16, kind="ExternalOutput")

    OFFS = np.cumsum([0] + WIDTHS).tolist()   # bd offset per quarter

    with tile.TileContext(nc) as tc:
        with (
            tc.tile_pool(name="res", bufs=1) as res,
            tc.tile_pool(name="pp", bufs=3) as ppool,
            tc.tile_pool(name="pg", bufs=2) as pgpool,
            tc.tile_pool(name="osb", bufs=4) as opool,
            tc.tile_pool(name="acc", bufs=8, space="PSUM") as apool,
        ):
            ftd_sb = res.tile([128, IB, BD], BF16)
            gtd_sb = res.tile([128, JB, BD], BF16)
            wt_sb = res.tile([128, NCHUNK, 128], BF16)
            bias_sb = res.tile([128, 1], F32)
            wl_sb = res.tile([128, 128], BF16, name="wl")
            wr_sb = res.tile([128, 512], BF16, name="wr")

            # PE warm-up: garbage matmuls on scratch tiles into the first
            # acc rotation slot keep the HAM clock busy during the DMA
            # ramp; bank is re-claimed by the last real accumulator long
            # after.  memset so CoreSim never sees uninitialized reads.
            nc.vector.memset(wl_sb[:], 0.0)
            nc.vector.memset(wr_sb[:], 0.0)
            warm = apool.tile([128, 512], F32, tag="acc")
            for k in range(NWARM):
                nc.tensor.matmul(warm[:], wl_sb[:], wr_sb[:],
                                 start=(k == 0), stop=(k == NWARM - 1))

            # streamed loads on one sync ring, strictly in first-need
            # order; h0 (512 wide) split per-jb/per-ib so the very first
            # sub-mul is gated on ~0.25 MB; wt sliced to 4-chunk pieces
            # interleaved with the ftd slices that pace the h0 muls; bias
            # alone on the ACT ring.  NO touches here -- they happen on
            # the consumer engine right before first use.
            W0 = WIDTHS[0]
            nc.scalar.dma_start(out=bias_sb[:], in_=bias[:])
            nc.sync.dma_start(out=gtd_sb[:, 0, 0:W0], in_=gtd[:, 0, 0:W0])
            nc.sync.dma_start(out=ftd_sb[:, 0, 0:W0], in_=ftd[:, 0, 0:W0])
            nc.sync.dma_start(out=gtd_sb[:, 1:JB, 0:W0],
                              in_=gtd[:, 1:JB, 0:W0])
            wt_cuts = [0, 4, 8, 12, 16, 24, 32, 32, 32]
            for ib in range(1, IB):
                lo, hi = wt_cuts[ib - 1], wt_cuts[ib]
                if hi > lo:
                    nc.sync.dma_start(out=wt_sb[:, lo:hi, :],
                                      in_=wt[:, lo:hi, :])
                nc.sync.dma_start(out=ftd_sb[:, ib, 0:W0],
                                  in_=ftd[:, ib, 0:W0])
            for h in range(1, NQ):
                lo, hi = OFFS[h], OFFS[h + 1]
                nc.sync.dma_start(out=gtd_sb[:, :, lo:hi],
                                  in_=gtd[:, :, lo:hi])
                nc.sync.dma_start(out=ftd_sb[:, :, lo:hi],
                                  in_=ftd[:, :, lo:hi])

            for h in range(NQ):
                lo, w = OFFS[h], WIDTHS[h]
                nbank = w // 512
                accs = [apool.tile([128, 512], F32, tag="acc",
                                   name=f"acc_{h}_{q}")
                        for q in range(nbank)]
                gsl = gtd_sb[:, :, lo:lo + w]
                if h > 0:
                    nc.vector.tensor_copy(gtd_sb[0:1, 0, lo:lo + 1],
                                          gtd_sb[0:1, 0, lo:lo + 1])
                    nc.vector.tensor_copy(ftd_sb[0:1, 0, lo:lo + 1],
                                          ftd_sb[0:1, 0, lo:lo + 1])

                ib_dve = IB - 1 if GP_OFFLOAD else IB
                pgt = None
                if GP_OFFLOAD:
                    # GpSimd computes the last i-block's products for this
                    # quarter concurrently with the DVE muls.  Its own
                    # touch chain (on bytes disjoint from the vector
                    # touches) keeps every instruction single-wait and
                    # keeps DVE independent of the GpSimd queue.
                    pgt = pgpool.tile([128, JB, w], BF16, tag="pg")
                    fap7 = (ftd_sb[:, IB - 1, lo:lo + w][:, None, :]
                            .to_broadcast((128, JB, w)))
                    nc.gpsimd.memset(pgt[0:1, 0:1, 0:2].bitcast(F32), 0.0)
                    nc.gpsimd.tensor_copy(pgt[0:1, 0:1, 0:1],
                                          gtd_sb[0:1, 1, lo:lo + 1])
                    nc.gpsimd.tensor_copy(pgt[0:1, 1:2, 0:1],
                                          ftd_sb[0:1, IB - 1, lo:lo + 1])
                    nc.gpsimd.tensor_mul(pgt[:], gsl, fap7)

                for ib in range(ib_dve):
                    if h == 0:
                        nc.vector.tensor_copy(ftd_sb[0:1, ib, 0:1],
                                              ftd_sb[0:1, ib, 0:1])
                    p2 = ppool.tile([128, JB, w], BF16, tag="p")
                    fap = (ftd_sb[:, ib, lo:lo + w][:, None, :]
                           .to_broadcast((128, JB, w)))
                    if h == 0 and ib == 0:
                        # per-jb sub-muls, each gated on one small load
                        for jb in range(JB):
                            nc.vector.tensor_copy(gtd_sb[0:1, jb, 0:1],
                                                  gtd_sb[0:1, jb, 0:1])
                            nc.vector.tensor_mul(
                                p2[:, jb:jb + 1, :],
                                gsl[:, jb:jb + 1, :],
                                fap[:, 0:1, :])
                    else:
                        nc.vector.tensor_mul(p2[:], gsl, fap)
                    for jb in range(JB):
                        c = JB * ib + jb
                        st = (ib == 0 and jb == 0)
                        sp = (ib == IB - 1 and jb == JB - 1)
                        for q in range(nbank):
                            nc.tensor.matmul(
                                accs[q][:], wt_sb[:, c, :],
                                p2[:, jb, q * 512:(q + 1) * 512],
                                start=st, stop=sp)
                if GP_OFFLOAD:
                    for jb in range(JB):
                        c = JB * (IB - 1) + jb
                        sp = (jb == JB - 1)
                        for q in range(nbank):
                            nc.tensor.matmul(
                                accs[q][:], wt_sb[:, c, :],
                                pgt[:, jb, q * 512:(q + 1) * 512],
                                start=False, stop=sp)
                for q in range(nbank):
                    osb = opool.tile([128, 512], BF16, tag="osb")
                    nc.scalar.activation(osb[:], accs[q][:],
                                         mybir.ActivationFunctionType.Identity,
                                         bias=bias_sb[:, 0:1])
                    nc.sync.dma_start(
                        out=out[:, lo + q * 512: lo + (q + 1) * 512],
                        in_=osb[:])
                    nc.vector.tensor_copy(osb[0:1, 0:1], osb[0:1, 0:1])
    if strip:
        _strip_self_waits(nc)
    return nc


def _get_nc() -> bass.Bass:
    if "nc" not in _CACHE:
        _CACHE["nc"] = _build_nc()
    return _CACHE["nc"]


def _prep_core_inputs(feature, base, W, b, ci):
    bsl = slice(ci * BLOC, (ci + 1) * BLOC)
    F = np.asarray(feature[bsl], np.float32)  # (128, 64, 32)
    G = np.asarray(base[bsl], np.float32)     # (128, 64, 32)

    fT = np.transpose(F, (1, 0, 2)).reshape(HK, BD)   # (i, bd)
    gT = np.transpose(G, (1, 0, 2)).reshape(H0, BD)   # (j, bd)

    # ftd[p=(16a+b), ib, bd] = fT[8*ib + a, bd]   (dup x16 over b)
    t = fT.reshape(IB, 8, BD).transpose(1, 0, 2)             # (a, ib, bd)
    ftd = np.broadcast_to(t[:, None], (8, 16, IB, BD))
    ftd = ftd.reshape(128, IB, BD)

    # gtd[p=(16a+b), jb, bd] = gT[16*jb + b, bd]   (dup x8 over a)
    t = gT.reshape(JB, 16, BD).transpose(1, 0, 2)            # (b, jb, bd)
    gtd = np.broadcast_to(t[None], (8, 16, JB, BD))
    gtd = gtd.reshape(128, JB, BD)

    # wt[p=(16a+b), c=(4*ib+jb), o] = W[o, (8*ib+a)*64 + 16*jb + b]
    wt = np.asarray(W, np.float32).reshape(O, IB, 8, JB, 16)
    wt = wt.transpose(2, 4, 1, 3, 0).reshape(128, NCHUNK, O)

    return {
        "ftd": np.ascontiguousarray(ftd).astype(BF),
        "gtd": np.ascontiguousarray(gtd).astype(BF),
        "wt": np.ascontiguousarray(wt).astype(BF),
        "bias": np.ascontiguousarray(b, np.float32).reshape(128, 1),
    }


def run(feature, base, W, b, **spmd_kwargs):
    nc = _get_nc()
    in_maps = [_prep_core_inputs(feature, base, W, b, ci) for ci in range(NCORES)]
    res = run_bass_kernel_spmd(nc, in_maps, list(range(NCORES)), **spmd_kwargs)
    outs = []
    for ci in range(NCORES):
        o = np.asarray(res.results[ci]["out"], dtype=np.float32)
        o = o.reshape(O, BLOC, D)
        outs.append(np.transpose(o, (1, 0, 2)))
    full = np.concatenate(outs, 0)
    return full, res


def kernel(feature, base, W, b):
    full, _ = run(feature, base, W, b)
    return full


# revision 14
# speedup vs baseline: 1.2287x; 1.2287x over previous
"""Trainium2 Bass kernel v3 for nn_CINComp.

out[b,o,d] = sum_{i,j} W[o,i*64+j]*feature[b,i,d]*base[b,j,d] + bias[o]

Data-parallel over batch B=1024 across 8 cores (BLOC=128 b/core).

v3 design ("dup-layout"): chunk the ij=4096 contraction into 32 chunks of
128 = (8 i's x 16 j's). Host-side duplicates BOTH factors to the chunk
partition layout p=(a,b): ftd dups each f-row x16 (8 MB/core), gtd dups
each g-row x8 (4 MB/core). The P=f*g product is then ONE unit-stride
bf16 SBUF DVE tensor_mul per (bd-quarter, i-block) at 2x mode -- no PE
broadcast matmuls, no PSUM intermediate, no ScalarE casts (vs v2 which
spent ~100us PE + ~90us ACT on that). PE does only the 256 main
contraction matmuls (~55us); DVE (~70us) is the pacer.

  - bd=4096 per core split into 4 quarters of 1024; PSUM = 8 banks = 4
    quarters x 2 accumulators [128,512], drained (bias add + bf16 cast)
    on ScalarE per quarter.
  - ~15 MB/core HBM traffic streamed in h-major slices so compute
    starts after ~1.3 MB.
  - dummy matmuls on scratch tiles during the DMA ramp keep/get the PE
    HAM clock warm (2.4 GHz) before the first real matmul.
"""

import numpy as np
import ml_dtypes

import concourse.bass as bass
import concourse.mybir as mybir
import concourse.tile as tile
from concourse.bass_utils import run_bass_kernel_spmd

B, HK, H0, D, O = 1024, 64, 64, 32, 128
NCORES = 8
BLOC = B // NCORES          # 128 batches per core
BD = BLOC * D               # 4096 bd points per core
WIDTHS = [512, 1536, 1536, 512]   # bd split; 512-multiples, sum=BD,
NQ = len(WIDTHS)                  # sum(w//512)=8 PSUM banks
IB = 8                      # i-blocks (8 i's each) -> ftd dup x16
JB = 4                      # j-blocks (16 j's each) -> gtd dup x8
NCHUNK = IB * JB            # 32 chunks of 128 = (a,b) partitions
NWARM = 6                   # PE warm-up matmuls during DMA ramp
GP_OFFLOAD = False          # GpSimd TT contends for the DVE SBUF port --
                            # measured +17 ns/elem on every DVE mul; keep off
F32 = mybir.dt.float32
BF16 = mybir.dt.bfloat16
BF = ml_dtypes.bfloat16

_CACHE = {}


def _strip_self_waits(nc: bass.Bass) -> None:
    """Transitively-minimal semaphore waits (see v2 docstring)."""
    UPD = ("sem-inc", "sem-add-imm")
    insts = [i for bb in nc.m.functions[0].blocks for i in bb.instructions]

    bad_sems = set()
    for i in insts:
        si = getattr(i, "sync_info", None)
        if si is None:
            continue
        for u in si.on_update:
            if u.sync_type != "semaphore" or u.update_mode not in UPD:
                bad_sems.add(u.id)

    def fifo_of(i):
        si = i.sync_info
        eng = str(getattr(i, "engine", None))
        if type(i).__name__ == "InstDMACopy" and si is not None:
            for u in si.on_update:
                if u.sync_type == "semaphore" and u.update_mode in UPD:
                    return ("q", u.id)
        return ("e", eng)

    cum: dict = {}
    event: dict = {}
    fifo_pred: dict = {}
    last_in_fifo: dict = {}
    metas = []
    for idx, i in enumerate(insts):
        si = getattr(i, "sync_info", None)
        f = fifo_of(i)
        fifo_pred[idx] = last_in_fifo.get(f)
        last_in_fifo[f] = idx
        ups = []
        if si is not None:
            for u in si.on_update:
                if u.sync_type == "semaphore" and u.update_mode in UPD:
                    cum[u.id] = cum.get(u.id, 0) + u.update_value
                    event[(u.id, cum[u.id])] = idx
                    ups.append((u.id, cum[u.id]))
        metas.append((si, ups))

    def resolve(sem, k):
        v = k
        while (sem, v) not in event:
            v += 1
            if v > cum.get(sem, 0):
                return None
        return event[(sem, v)]

    cvc: list = [None] * len(insts)

    def get_cvc(idx):
        if cvc[idx] is not None:
            return cvc[idx]
        stack = [idx]
        while stack:
            j = stack[-1]
            if cvc[j] is not None:
                stack.pop()
                continue
            si, ups = metas[j]
            deps = []
            p = fifo_pred[j]
            if p is not None:
                deps.append(p)
            if si is not None:
                for w in si.on_wait:
                    if (
                        w.sync_type == "semaphore"
                        and w.wait_mode == "sem-ge-imm"
                        and w.id not in bad_sems
                    ):
                        e = resolve(w.id, w.wait_value)
                        if e is not None and e != j:
                            deps.append(e)
            pending = [d for d in deps if cvc[d] is None]
            if pending:
                stack.extend(pending)
                continue
            stack.pop()
            vc: dict = {}
            for d in deps:
                for s, v in cvc[d].items():
                    if vc.get(s, 0) < v:
                        vc[s] = v
            if si is not None:
                for w in si.on_wait:
                    if (
                        w.sync_type == "semaphore"
                        and w.wait_mode == "sem-ge-imm"
                        and w.id not in bad_sems
                    ):
                        if vc.get(w.id, 0) < w.wait_value:
                            vc[w.id] = w.wait_value
            for s, v in ups:
                if vc.get(s, 0) < v:
                    vc[s] = v
            cvc[j] = vc
        return cvc[idx]

    for idx, i in enumerate(insts):
        si, _ups = metas[idx]
        if si is None or not si.on_wait:
            continue
        base: dict = {}
        p = fifo_pred[idx]
        if p is not None:
            base = dict(get_cvc(p))
        sem_waits = [
            w
            for w in si.on_wait
            if w.sync_type == "semaphore"
            and w.wait_mode == "sem-ge-imm"
            and w.id not in bad_sems
        ]
        other = [w for w in si.on_wait if w not in sem_waits]

        def strength(w):
            e = resolve(w.id, w.wait_value)
            return len(get_cvc(e)) if e is not None else 0

        sem_waits.sort(key=strength, reverse=True)

        def wait_cvc(w):
            e = resolve(w.id, w.wait_value)
            vc = dict(get_cvc(e)) if e is not None else {}
            if vc.get(w.id, 0) < w.wait_value:
                vc[w.id] = w.wait_value
            return vc

        kept = sem_waits[:]
        changed = True
        while changed:
            changed = False
            for w in kept:
                cover = dict(base)
                for w2 in kept:
                    if w2 is w:
                        continue
                    for s, v in wait_cvc(w2).items():
                        if cover.get(s, 0) < v:
                            cover[s] = v
                if cover.get(w.id, 0) >= w.wait_value:
                    kept.remove(w)
                    changed = True
                    break
        if len(kept) + len(other) != len(si.on_wait):
            si.on_wait = other + kept


def _build_nc(strip: bool = True) -> bass.Bass:
    nc = bass.Bass()
    ftd = nc.dram_tensor("ftd", [128, IB, BD], BF16, kind="ExternalInput")
    gtd = nc.dram_tensor("gtd", [128, JB, BD], BF16, kind="ExternalInput")
    wt = nc.dram_tensor("wt", [128, NCHUNK, 128], BF16, kind="ExternalInput")
    bias = nc.dram_tensor("bias", [128, 1], F32, kind="ExternalInput")
    out = nc.dram_tensor("out", [128, BD], BF16, kind="ExternalOutput")

    OFFS = np.cumsum([0] + WIDTHS).tolist()   # bd offset per quarter

    with tile.TileContext(nc) as tc:
        with (
            tc.tile_pool(name="res", bufs=1) as res,
            tc.tile_pool(name="pp", bufs=3) as ppool,
            tc.tile_pool(name="pg", bufs=2) as pgpool,
            tc.tile_pool(name="osb", bufs=4) as opool,
            tc.tile_pool(name="acc", bufs=8, space="PSUM") as apool,
        ):
            ftd_sb = res.tile([128, IB, BD], BF16)
            gtd_sb = res.tile([128, JB, BD], BF16)
            wt_sb = res.tile([128, NCHUNK, 128], BF16)
            bias_sb = res.tile([128, 1], F32)
            wl_sb = res.tile([128, 128], BF16, name="wl")
            wr_sb = res.tile([128, 512], BF16, name="wr")

            # PE warm-up: garbage matmuls on scratch tiles into the first
            # acc rotation slot keep the HAM clock busy during the DMA
            # ramp; bank is re-claimed by the last real accumulator long
            # after.  memset so CoreSim never sees uninitialized reads.
            nc.vector.memset(wl_sb[:], 0.0)
            nc.vector.memset(wr_sb[:], 0.0)
            warm = apool.tile([128, 512], F32, tag="acc")
            for k in range(NWARM):
                nc.tensor.matmul(warm[:], wl_sb[:], wr_sb[:],
                                 start=(k == 0), stop=(k == NWARM - 1))

            # streamed loads on one sync ring, strictly in first-need
            # order; h0 (512 wide) split per-jb/per-ib so the very first
            # sub-mul is gated on ~0.25 MB; wt sliced to 4-chunk pieces
            # interleaved with the ftd slices that pace the h0 muls; bias
            # alone on the ACT ring.  NO touches here -- they happen on
            # the consumer engine right before first use.
            W0 = WIDTHS[0]
            nc.scalar.dma_start(out=bias_sb[:], in_=bias[:])
            nc.sync.dma_start(out=gtd_sb[:, 0, 0:W0], in_=gtd[:, 0, 0:W0])
            nc.sync.dma_start(out=ftd_sb[:, 0, 0:W0], in_=ftd[:, 0, 0:W0])
            nc.sync.dma_start(out=gtd_sb[:, 1:JB, 0:W0],
                              in_=gtd[:, 1:JB, 0:W0])
            wt_cuts = [0, 4, 8, 12, 16, 24, 32, 32, 32]
            for ib in range(1, IB):
                lo, hi = wt_cuts[ib - 1], wt_cuts[ib]
                if hi > lo:
                    nc.sync.dma_start(out=wt_sb[:, lo:hi, :],
                                      in_=wt[:, lo:hi, :])
                nc.sync.dma_start(out=ftd_sb[:, ib, 0:W0],
                                  in_=ftd[:, ib, 0:W0])
            for h in range(1, NQ):
                lo, hi = OFFS[h], OFFS[h + 1]
                nc.sync.dma_start(out=gtd_sb[:, :, lo:hi],
                                  in_=gtd[:, :, lo:hi])
                nc.sync.dma_start(out=ftd_sb[:, :, lo:hi],
                                  in_=ftd[:, :, lo:hi])

            for h in range(NQ):
                lo, w = OFFS[h], WIDTHS[h]
                nbank = w // 512
                accs = [apool.tile([128, 512], F32, tag="acc",
                                   name=f"acc_{h}_{q}")
                        for q in range(nbank)]
                gsl = gtd_sb[:, :, lo:lo + w]
                if h > 0:
                    nc.vector.tensor_copy(gtd_sb[0:1, 0, lo:lo + 1],
                                          gtd_sb[0:1, 0, lo:lo + 1])
                    nc.vector.tensor_copy(ftd_sb[0:1, 0, lo:lo + 1],
                                          ftd_sb[0:1, 0, lo:lo + 1])

                ib_dve = IB - 1 if GP_OFFLOAD else IB
                pgt = None
                if GP_OFFLOAD:
                    # GpSimd computes the last i-block's products for this
                    # quarter concurrently with the DVE muls.  Its own
                    # touch chain (on bytes disjoint from the vector
                    # touches) keeps every instruction single-wait and
                    # keeps DVE independent of the GpSimd queue.
                    pgt = pgpool.tile([128, JB, w], BF16, tag="pg")
                    fap7 = (ftd_sb[:, IB - 1, lo:lo + w][:, None, :]
                            .to_broadcast((128, JB, w)))
                    nc.gpsimd.memset(pgt[0:1, 0:1, 0:2].bitcast(F32), 0.0)
                    nc.gpsimd.tensor_copy(pgt[0:1, 0:1, 0:1],
                                          gtd_sb[0:1, 1, lo:lo + 1])
                    nc.gpsimd.tensor_copy(pgt[0:1, 1:2, 0:1],
                                          ftd_sb[0:1, IB - 1, lo:lo + 1])
                    nc.gpsimd.tensor_mul(pgt[:], gsl, fap7)

                for ib in range(ib_dve):
                    if h == 0:
                        nc.vector.tensor_copy(ftd_sb[0:1, ib, 0:1],
                                              ftd_sb[0:1, ib, 0:1])
                    p2 = ppool.tile([128, JB, w], BF16, tag="p")
                    fap = (ftd_sb[:, ib, lo:lo + w][:, None, :]
                           .to_broadcast((128, JB, w)))
                    if h == 0 and ib == 0:
                        # per-jb sub-muls, each gated on one small load
                        for jb in range(JB):
                            nc.vector.tensor_copy(gtd_sb[0:1, jb, 0:1],
                                                  gtd_sb[0:1, jb, 0:1])
                            nc.vector.tensor_mul(
                                p2[:, jb:jb + 1, :],
                                gsl[:, jb:jb + 1, :],
                                fap[:, 0:1, :])
                    else:
                        nc.vector.tensor_mul(p2[:], gsl, fap)
                    for jb in range(JB):
                        c = JB * ib + jb
                        st = (ib == 0 and jb == 0)
                        sp = (ib == IB - 1 and jb == JB - 1)
                        for q in range(nbank):
                            nc.tensor.matmul(
                                accs[q][:], wt_sb[:, c, :],
                                p2[:, jb, q * 512:(q + 1) * 512],
                                start=st, stop=sp)
                if GP_OFFLOAD:
                    for jb in range(JB):
                        c = JB * (IB - 1) + jb
                        sp = (jb == JB - 1)
                        for q in range(nbank):
                            nc.tensor.matmul(
                                accs[q][:], wt_sb[:, c, :],
                                pgt[:, jb, q * 512:(q + 1) * 512],
                                start=False, stop=sp)
                for q in range(nbank):
                    osb = opool.tile([128, 512], BF16, tag="osb")
                    nc.scalar.activation(osb[:], accs[q][:],
                                         mybir.ActivationFunctionType.Identity,
                                         bias=bias_sb[:, 0:1])
                    nc.sync.dma_start(
                        out=out[:, lo + q * 512: lo + (q + 1) * 512],
                        in_=osb[:])
                    nc.vector.tensor_copy(osb[0:1, 0:1], osb[0:1, 0:1])
    if strip:
        _strip_self_waits(nc)
    return nc


def _get_nc() -> bass.Bass:
    if "nc" not in _CACHE:
        _CACHE["nc"] = _build_nc()
    return _CACHE["nc"]


def _prep_core_inputs(feature, base, W, b, ci):
    bsl = slice(ci * BLOC, (ci + 1) * BLOC)
    F = np.asarray(feature[bsl], np.float32)  # (128, 64, 32)
    G = np.asarray(base[bsl], np.float32)     # (128, 64, 32)

    fT = np.transpose(F, (1, 0, 2)).reshape(HK, BD)   # (i, bd)
    gT = np.transpose(G, (1, 0, 2)).reshape(H0, BD)   # (j, bd)

    # ftd[p=(16a+b), ib, bd] = fT[8*ib + a, bd]   (dup x16 over b)
    t = fT.reshape(IB, 8, BD).transpose(1, 0, 2)             # (a, ib, bd)
    ftd = np.broadcast_to(t[:, None], (8, 16, IB, BD))
    ftd = ftd.reshape(128, IB, BD)

    # gtd[p=(16a+b), jb, bd] = gT[16*jb + b, bd]   (dup x8 over a)
    t = gT.reshape(JB, 16, BD).transpose(1, 0, 2)            # (b, jb, bd)
    gtd = np.broadcast_to(t[None], (8, 16, JB, BD))
    gtd = gtd.reshape(128, JB, BD)

    # wt[p=(16a+b), c=(4*ib+jb), o] = W[o, (8*ib+a)*64 + 16*jb + b]
    wt = np.asarray(W, np.float32).reshape(O, IB, 8, JB, 16)
    wt = wt.transpose(2, 4, 1, 3, 0).reshape(128, NCHUNK, O)

    return {
        "ftd": np.ascontiguousarray(ftd).astype(BF),
        "gtd": np.ascontiguousarray(gtd).astype(BF),
        "wt": np.ascontiguousarray(wt).astype(BF),
        "bias": np.ascontiguousarray(b, np.float32).reshape(128, 1),
    }


def run(feature, base, W, b, **spmd_kwargs):
    nc = _get_nc()
    in_maps = [_prep_core_inputs(feature, base, W, b, ci) for ci in range(NCORES)]
    res = run_bass_kernel_spmd(nc, in_maps, list(range(NCORES)), **spmd_kwargs)
    outs = []
    for ci in range(NCORES):
        o = np.asarray(res.results[ci]["out"], dtype=np.float32)
        o = o.reshape(O, BLOC, D)
        outs.append(np.transpose(o, (1, 0, 2)))
    full = np.concatenate(outs, 0)
    return full, res


def kernel(feature, base, W, b):
    full, _ = run(feature, base, W, b)
    return full
